# revision 17
# baseline (speedup 1.0000x reference)
"""ContrastiveCenterLoss kernel for 8x Trainium2 NeuronCores (Bass/Tile).

Math (matches the reference):
    hist = bincount(y, C) ; count = hist + 1
    dist_i = ||hidden_i - centers[y_i]||^2
    s = sum_i dist_i / count[y_i]
      = sum_c D_c / (hist_c + 1),   D_c = sum_{i: y_i = c} dist_i
    loss = 0.5 * s / (s + 1e-6)

Strategy: data-parallel over the batch (8192 samples/core).  Each core:
  - streams its hidden shard (f32) on the two HWDGE rings (SP + Act)
  - dma_gather's per-sample center rows from a host-precast bf16 table
    (gathers ride 4 SWDGE queues, single_packet descriptor bursts)
  - per 128-sample tile: DVE subtract, ACT square+accumulate -> dist
  - bins (dist, 1) per class with a hi/lo class-id split: two tiny
    one-hots [128,8]/[128,128] and one PE matmul accumulating a
    [16,128] f32 PSUM tile (rows 0:8 = D bins, rows 8:16 = hist bins)
Host combines the 8 per-core [16,128] partials: s = sum D/(hist+1),
loss = 0.5*s/(s+eps).
"""

import numpy as np

B = 65536
D = 512
C = 1000
NCORES = 8
BLOC = B // NCORES          # 8192 samples per core
P = 128                     # partitions
SLOTS = BLOC // P           # 64 sample-slots per partition
PLAN = [(i * 4, 4) for i in range(16)]  # (start_slot, n_slots)
CHUNKS = len(PLAN)
QPC = 4                     # max slots per chunk (tile sizing)
HI = 8                      # class-id high part (c >> 7), 0..7
LO = 128                    # class-id low part (c & 127)
LAMBDA_C = 1.0
EPS = 1e-6

_CACHE = {}


def _build_program():
    import concourse.bacc as bacc
    import concourse.tile as tile
    from concourse import library_config, mybir

    f32 = mybir.dt.float32
    i32 = mybir.dt.int32
    f16 = mybir.dt.bfloat16
    i16 = mybir.dt.int16
    Alu = mybir.AluOpType
    Act = mybir.ActivationFunctionType

    nc = bacc.Bacc(
        "TRN2",
        target_bir_lowering=False,
        debug=False,
        enable_asserts=False,
        num_devices=NCORES,
        num_swdge_queues=4,
        dynamic_dma_scratch_size=65536,
    )

    hidden_ap = nc.dram_tensor("hidden", [BLOC, D], f32, kind="ExternalInput").ap()
    ctab_ap = nc.dram_tensor("ctab16", [C, D], f16, kind="ExternalInput").ap()
    # ypg(64) | clslo(128) | clshi(8), one DMA for all the small inputs
    pk_ap = nc.dram_tensor("pk", [P, 200], i32, kind="ExternalInput").ap()
    yidx_ap = nc.dram_tensor("yidx", [P, 8 * SLOTS], i16, kind="ExternalInput").ap()
    out_ap = nc.dram_tensor("out", [2 * HI, LO], f32, kind="ExternalOutput").ap()

    # sample (p, t) of this core's shard is shard row p*SLOTS + t
    hview = hidden_ap.rearrange("(p t) d -> p t d", p=P)

    with tile.TileContext(nc) as tc:
        with (
            tc.tile_pool(name="persist", bufs=1) as persist,
            tc.tile_pool(name="hpool", bufs=4) as hpool,
            tc.tile_pool(name="cpool", bufs=4) as cpool,
            tc.tile_pool(name="dpool", bufs=2) as dpool,
            tc.tile_pool(name="spool", bufs=4) as spool,
            tc.tile_pool(name="lpool", bufs=4) as lpool,
            tc.tile_pool(name="opool", bufs=4) as opool,
            tc.tile_pool(name="psum", bufs=1, space="PSUM") as psum,
        ):
            nc.gpsimd.load_library(library_config.mlp)
            # small inputs first on the SP ring so they are at the FIFO
            # front, ahead of the bulk hidden stream
            pk = persist.tile([P, 200], i32)
            nc.sync.dma_start(pk[:], pk_ap[:])
            yidx = persist.tile([P, 8 * SLOTS], i16)
            nc.sync.dma_start(yidx[:], yidx_ap[:])

            # y -> (hi, lo) split, as f16 per-partition scalar sources
            ylo_i = persist.tile([P, SLOTS], i32)
            nc.vector.tensor_scalar(ylo_i[:], pk[:, :SLOTS], 127, None, op0=Alu.bitwise_and)
            yhi_i = persist.tile([P, SLOTS], i32)
            nc.vector.tensor_scalar(
                yhi_i[:], pk[:, :SLOTS], 7, None, op0=Alu.logical_shift_right
            )
            ylo = persist.tile([P, SLOTS], f16)
            nc.vector.tensor_copy(ylo[:], ylo_i[:])
            yhi = persist.tile([P, SLOTS], f16)
            nc.vector.tensor_copy(yhi[:], yhi_i[:])

            # class-id iotas from the packed host input (exact in bf16)
            cls_lo = persist.tile([P, LO], f16)
            nc.vector.tensor_copy(cls_lo[:], pk[:, SLOTS : SLOTS + LO])
            cls_hi = persist.tile([P, HI], f16)
            nc.vector.tensor_copy(cls_hi[:], pk[:, SLOTS + LO :])

            dist = persist.tile([P, SLOTS], f32)
            acc = psum.tile([2 * HI, LO], f32)

            for g, (s0, ns) in enumerate(PLAN):
                sl = slice(s0, s0 + ns)
                h32 = hpool.tile([P, QPC, D], f32)
                heng = nc.sync if g % 2 == 0 else nc.scalar
                heng.dma_start(h32[:, :ns, :], hview[:, sl, :])
                c16 = cpool.tile([P, QPC, D], f16)
                nc.gpsimd.dma_gather(
                    c16[:, :ns, :],
                    ctab_ap[:],
                    yidx[:, 8 * s0 : 8 * (s0 + ns)],
                    num_idxs=P * ns,
                    num_idxs_reg=P * ns,
                    elem_size=D,
                    single_packet=True,
                    queue_num=g % 4,
                )

                # whole-chunk subtract (one DVE op); h f32, c bf16, out bf16
                diff = dpool.tile([P, QPC, D], f16)
                nc.vector.tensor_sub(
                    diff[:, :ns, :].rearrange("p q d -> p (q d)"),
                    h32[:, :ns, :].rearrange("p q d -> p (q d)"),
                    c16[:, :ns, :].rearrange("p q d -> p (q d)"),
                )

                # per-tile square + accumulate -> dist column (ACT)
                for q in range(ns):
                    t = s0 + q
                    sq = spool.tile([P, D], f16)
                    nc.scalar.activation(
                        sq[:], diff[:, q, :], Act.Square,
                        accum_out=dist[:, t : t + 1],
                    )

                # batched one-hots for the whole chunk
                lhsT = lpool.tile([P, QPC, 2 * HI], f16)
                nc.vector.tensor_tensor(
                    lhsT[:, :ns, HI:],
                    cls_hi[:].unsqueeze(1).to_broadcast([P, ns, HI]),
                    yhi[:, sl].unsqueeze(2).to_broadcast([P, ns, HI]),
                    op=Alu.is_equal,
                )
                nc.vector.tensor_tensor(
                    lhsT[:, :ns, :HI],
                    lhsT[:, :ns, HI:],
                    dist[:, sl].unsqueeze(2).to_broadcast([P, ns, HI]),
                    op=Alu.mult,
                )
                ohlo = opool.tile([P, QPC, LO], f16)
                nc.vector.tensor_tensor(
                    ohlo[:, :ns, :],
                    cls_lo[:].unsqueeze(1).to_broadcast([P, ns, LO]),
                    ylo[:, sl].unsqueeze(2).to_broadcast([P, ns, LO]),
                    op=Alu.is_equal,
                )

                for q in range(ns):
                    t = s0 + q
                    nc.tensor.matmul(
                        out=acc[:],
                        lhsT=lhsT[:, q, :],
                        rhs=ohlo[:, q, :],
                        start=(t == 0),
                        stop=(t == SLOTS - 1),
                    )

            res = persist.tile([2 * HI, LO], f32)
            nc.vector.tensor_copy(res[:], acc[:])
            nc.scalar.dma_start(out_ap[:], res[:])

    nc.compile()
    return nc


def _prep_core_inputs(y_shard, hidden_shard, ctab16):
    """Host-side layout marshaling for one core's shard."""
    ypg = y_shard.astype(np.int32).reshape(P, SLOTS)  # sample (p,t) = row p*SLOTS+t

    # dma_gather idx list for chunk g, position j = q*128 + p -> sample
    # (p, g*QPC+q); wrapped: idx j lives at [j % 16, j // 16], replicated
    # over the 8 groups of 16 partitions.
    cols = []
    for s0, ns in PLAN:
        flat = ypg[:, s0 : s0 + ns].T.reshape(P * ns)
        wrapped = flat.reshape(P * ns // 16, 16).T
        cols.append(np.tile(wrapped, (P // 16, 1)))
    yidx = np.concatenate(cols, axis=1).astype(np.int16)

    pk = np.empty((P, 200), dtype=np.int32)
    pk[:, :SLOTS] = ypg
    pk[:, SLOTS : SLOTS + LO] = np.arange(LO, dtype=np.int32)
    pk[:, SLOTS + LO :] = np.arange(HI, dtype=np.int32)

    return {
        "hidden": np.ascontiguousarray(hidden_shard, dtype=np.float32),
        "ctab16": ctab16,
        "pk": pk,
        "yidx": np.ascontiguousarray(yidx),
    }


def combine_partials(outs):
    """outs: list of [16, 128] f32 per core -> scalar loss (f32)."""
    total = np.zeros((2 * HI, LO), dtype=np.float64)
    for o in outs:
        total += o.astype(np.float64)
    Dbins = total[:HI].reshape(HI * LO)[:C]
    hist = total[HI:].reshape(HI * LO)[:C]
    s = float(np.sum(Dbins / (hist + 1.0)))
    try:
        # match the reference's XLA f32 division rounding exactly
        import jax.numpy as jnp

        s32 = jnp.float32(s)
        loss = jnp.float32(LAMBDA_C / 2.0) * s32 / (s32 + jnp.float32(EPS))
        return np.asarray(loss, dtype=np.float32)
    except Exception:
        return np.float32((LAMBDA_C / 2.0) * s / (s + EPS))


def kernel(y, hidden, centers):
    import ml_dtypes
    from concourse.bass_utils import run_bass_kernel_spmd

    y = np.asarray(y).astype(np.int32)
    hidden = np.asarray(hidden, dtype=np.float32)
    centers = np.asarray(centers, dtype=np.float32)
    ctab16 = np.ascontiguousarray(centers.astype(ml_dtypes.bfloat16))

    if "nc" not in _CACHE:
        _CACHE["nc"] = _build_program()
    nc = _CACHE["nc"]

    in_maps = [
        _prep_core_inputs(
            y[k * BLOC : (k + 1) * BLOC],
            hidden[k * BLOC : (k + 1) * BLOC],
            ctab16,
        )
        for k in range(NCORES)
    ]

    res = run_bass_kernel_spmd(nc, in_maps, core_ids=list(range(NCORES)))
    outs = [r["out"] for r in res.results]
    return combine_partials(outs)


# revision 22
# speedup vs baseline: 1.0735x; 1.0735x over previous
"""ContrastiveCenterLoss kernel for 8x Trainium2 NeuronCores (Bass/Tile).

Math (matches the reference):
    hist = bincount(y, C) ; count = hist + 1
    dist_i = ||hidden_i - centers[y_i]||^2
    s = sum_i dist_i / count[y_i]
      = sum_c D_c / (hist_c + 1),   D_c = sum_{i: y_i = c} dist_i
    loss = 0.5 * s / (s + 1e-6)

Strategy: data-parallel over the batch (8192 samples/core).  Each core:
  - streams its hidden shard (f32) on the two HWDGE rings (SP + Act)
  - dma_gather's per-sample center rows from a host-precast bf16 table
    (gathers ride 4 SWDGE queues, single_packet descriptor bursts)
  - per 128-sample tile: DVE subtract, ACT square+accumulate -> dist
  - bins (dist, 1) per class with a hi/lo class-id split: two tiny
    one-hots [128,8]/[128,128] and one PE matmul accumulating a
    [16,128] f32 PSUM tile (rows 0:8 = D bins, rows 8:16 = hist bins)
Host combines the 8 per-core [16,128] partials: s = sum D/(hist+1),
loss = 0.5*s/(s+eps).
"""

import numpy as np

B = 65536
D = 512
C = 1000
NCORES = 8
BLOC = B // NCORES          # 8192 samples per core
P = 128                     # partitions
SLOTS = BLOC // P           # 64 sample-slots per partition
PLAN = [(i * 4, 4) for i in range(16)]  # (start_slot, n_slots)
CHUNKS = len(PLAN)
QPC = 4                     # max slots per chunk (tile sizing)
HI = 8                      # class-id high part (c >> 7), 0..7
LO = 128                    # class-id low part (c & 127)
LAMBDA_C = 1.0
EPS = 1e-6

_CACHE = {}


def _build_program():
    import concourse.bacc as bacc
    import concourse.tile as tile
    from concourse import library_config, mybir

    f32 = mybir.dt.float32
    i32 = mybir.dt.int32
    f16 = mybir.dt.bfloat16
    f8 = mybir.dt.float8e4
    i16 = mybir.dt.int16
    Alu = mybir.AluOpType
    Act = mybir.ActivationFunctionType

    nc = bacc.Bacc(
        "TRN2",
        target_bir_lowering=False,
        debug=False,
        enable_asserts=False,
        num_devices=NCORES,
        num_swdge_queues=4,
        dynamic_dma_scratch_size=65536,
    )

    hidden_ap = nc.dram_tensor("hidden", [BLOC, D], f32, kind="ExternalInput").ap()
    ctab_ap = nc.dram_tensor("ctab16", [C, D], f8, kind="ExternalInput").ap()
    # ypg(64) | clslo(128) | clshi(8), one DMA for all the small inputs
    pk_ap = nc.dram_tensor("pk", [P, 200], i32, kind="ExternalInput").ap()
    yidx_ap = nc.dram_tensor("yidx", [P, 8 * SLOTS], i16, kind="ExternalInput").ap()
    out_ap = nc.dram_tensor("out", [2 * HI, LO], f32, kind="ExternalOutput").ap()

    # sample (p, t) of this core's shard is shard row p*SLOTS + t
    hview = hidden_ap.rearrange("(p t) d -> p t d", p=P)

    with tile.TileContext(nc) as tc:
        with (
            tc.tile_pool(name="persist", bufs=1) as persist,
            tc.tile_pool(name="hpool", bufs=10) as hpool,
            tc.tile_pool(name="cpool", bufs=6) as cpool,
            tc.tile_pool(name="dpool", bufs=2) as dpool,
            tc.tile_pool(name="spool", bufs=4) as spool,
            tc.tile_pool(name="lpool", bufs=4) as lpool,
            tc.tile_pool(name="opool", bufs=4) as opool,
            tc.tile_pool(name="psum", bufs=1, space="PSUM") as psum,
        ):
            nc.gpsimd.load_library(library_config.mlp)
            # small inputs first on the SP ring so they are at the FIFO
            # front, ahead of the bulk hidden stream
            pk = persist.tile([P, 200], i32)
            nc.sync.dma_start(pk[:], pk_ap[:])
            yidx = persist.tile([P, 8 * SLOTS], i16)
            nc.sync.dma_start(yidx[:], yidx_ap[:])

            # y -> (hi, lo) split, as f16 per-partition scalar sources
            ylo_i = persist.tile([P, SLOTS], i32)
            nc.vector.tensor_scalar(ylo_i[:], pk[:, :SLOTS], 127, None, op0=Alu.bitwise_and)
            yhi_i = persist.tile([P, SLOTS], i32)
            nc.vector.tensor_scalar(
                yhi_i[:], pk[:, :SLOTS], 7, None, op0=Alu.logical_shift_right
            )
            ylo = persist.tile([P, SLOTS], f16)
            nc.vector.tensor_copy(ylo[:], ylo_i[:])
            yhi = persist.tile([P, SLOTS], f16)
            nc.vector.tensor_copy(yhi[:], yhi_i[:])

            # class-id iotas from the packed host input (exact in bf16)
            cls_lo = persist.tile([P, LO], f16)
            nc.vector.tensor_copy(cls_lo[:], pk[:, SLOTS : SLOTS + LO])
            cls_hi = persist.tile([P, HI], f16)
            nc.vector.tensor_copy(cls_hi[:], pk[:, SLOTS + LO :])

            dist = persist.tile([P, SLOTS], f32)
            acc = psum.tile([2 * HI, LO], f32)

            for g, (s0, ns) in enumerate(PLAN):
                sl = slice(s0, s0 + ns)
                h32 = hpool.tile([P, QPC, D], f32)
                heng = nc.sync if g % 2 == 0 else nc.scalar
                heng.dma_start(h32[:, :ns, :], hview[:, sl, :])
                c16 = cpool.tile([P, QPC, D], f8)
                nc.gpsimd.dma_gather(
                    c16[:, :ns, :],
                    ctab_ap[:],
                    yidx[:, 8 * s0 : 8 * (s0 + ns)],
                    num_idxs=P * ns,
                    num_idxs_reg=P * ns,
                    elem_size=D,
                    single_packet=True,
                    queue_num=g % 4,
                )

                # whole-chunk subtract (one DVE op); h f32, c bf16, out bf16
                diff = dpool.tile([P, QPC, D], f16)
                nc.vector.tensor_sub(
                    diff[:, :ns, :].rearrange("p q d -> p (q d)"),
                    h32[:, :ns, :].rearrange("p q d -> p (q d)"),
                    c16[:, :ns, :].rearrange("p q d -> p (q d)"),
                )

                # per-tile square + accumulate -> dist column (ACT)
                for q in range(ns):
                    t = s0 + q
                    sq = spool.tile([P, D], f16)
                    nc.scalar.activation(
                        sq[:], diff[:, q, :], Act.Square,
                        accum_out=dist[:, t : t + 1],
                    )

                # batched one-hots for the whole chunk
                lhsT = lpool.tile([P, QPC, 2 * HI], f16)
                nc.vector.tensor_tensor(
                    lhsT[:, :ns, HI:],
                    cls_hi[:].unsqueeze(1).to_broadcast([P, ns, HI]),
                    yhi[:, sl].unsqueeze(2).to_broadcast([P, ns, HI]),
                    op=Alu.is_equal,
                )
                nc.vector.tensor_tensor(
                    lhsT[:, :ns, :HI],
                    lhsT[:, :ns, HI:],
                    dist[:, sl].unsqueeze(2).to_broadcast([P, ns, HI]),
                    op=Alu.mult,
                )
                ohlo = opool.tile([P, QPC, LO], f16)
                nc.vector.tensor_tensor(
                    ohlo[:, :ns, :],
                    cls_lo[:].unsqueeze(1).to_broadcast([P, ns, LO]),
                    ylo[:, sl].unsqueeze(2).to_broadcast([P, ns, LO]),
                    op=Alu.is_equal,
                )

                for q in range(ns):
                    t = s0 + q
                    nc.tensor.matmul(
                        out=acc[:],
                        lhsT=lhsT[:, q, :],
                        rhs=ohlo[:, q, :],
                        start=(t == 0),
                        stop=(t == SLOTS - 1),
                    )

            res = persist.tile([2 * HI, LO], f32)
            nc.vector.tensor_copy(res[:], acc[:])
            nc.scalar.dma_start(out_ap[:], res[:])

    nc.compile()
    return nc


def _prep_core_inputs(y_shard, hidden_shard, ctab16):
    """Host-side layout marshaling for one core's shard."""
    ypg = y_shard.astype(np.int32).reshape(P, SLOTS)  # sample (p,t) = row p*SLOTS+t

    # dma_gather idx list for chunk g, position j = q*128 + p -> sample
    # (p, g*QPC+q); wrapped: idx j lives at [j % 16, j // 16], replicated
    # over the 8 groups of 16 partitions.
    cols = []
    for s0, ns in PLAN:
        flat = ypg[:, s0 : s0 + ns].T.reshape(P * ns)
        wrapped = flat.reshape(P * ns // 16, 16).T
        cols.append(np.tile(wrapped, (P // 16, 1)))
    yidx = np.concatenate(cols, axis=1).astype(np.int16)

    pk = np.empty((P, 200), dtype=np.int32)
    pk[:, :SLOTS] = ypg
    pk[:, SLOTS : SLOTS + LO] = np.arange(LO, dtype=np.int32)
    pk[:, SLOTS + LO :] = np.arange(HI, dtype=np.int32)

    return {
        "hidden": np.ascontiguousarray(hidden_shard, dtype=np.float32),
        "ctab16": ctab16,
        "pk": pk,
        "yidx": np.ascontiguousarray(yidx),
    }


def combine_partials(outs):
    """outs: list of [16, 128] f32 per core -> scalar loss (f32)."""
    total = np.zeros((2 * HI, LO), dtype=np.float64)
    for o in outs:
        total += o.astype(np.float64)
    Dbins = total[:HI].reshape(HI * LO)[:C]
    hist = total[HI:].reshape(HI * LO)[:C]
    s = float(np.sum(Dbins / (hist + 1.0)))
    try:
        # match the reference's XLA f32 division rounding exactly
        import jax.numpy as jnp

        s32 = jnp.float32(s)
        loss = jnp.float32(LAMBDA_C / 2.0) * s32 / (s32 + jnp.float32(EPS))
        return np.asarray(loss, dtype=np.float32)
    except Exception:
        return np.float32((LAMBDA_C / 2.0) * s / (s + EPS))


def kernel(y, hidden, centers):
    import ml_dtypes
    from concourse.bass_utils import run_bass_kernel_spmd

    y = np.asarray(y).astype(np.int32)
    hidden = np.asarray(hidden, dtype=np.float32)
    centers = np.asarray(centers, dtype=np.float32)
    ctab16 = np.ascontiguousarray(centers.astype(ml_dtypes.float8_e4m3))

    if "nc" not in _CACHE:
        _CACHE["nc"] = _build_program()
    nc = _CACHE["nc"]

    in_maps = [
        _prep_core_inputs(
            y[k * BLOC : (k + 1) * BLOC],
            hidden[k * BLOC : (k + 1) * BLOC],
            ctab16,
        )
        for k in range(NCORES)
    ]

    res = run_bass_kernel_spmd(nc, in_maps, core_ids=list(range(NCORES)))
    outs = [r["out"] for r in res.results]
    return combine_partials(outs)


# revision 27
# speedup vs baseline: 1.1776x; 1.0970x over previous
"""ContrastiveCenterLoss kernel for 8x Trainium2 NeuronCores (Bass/Tile).

Math (matches the reference):
    hist = bincount(y, C) ; count = hist + 1
    dist_i = ||hidden_i - centers[y_i]||^2
    s = sum_i dist_i / count[y_i]
      = sum_c D_c / (hist_c + 1),   D_c = sum_{i: y_i = c} dist_i
    loss = 0.5 * s / (s + 1e-6)

Strategy: data-parallel over the batch (8192 samples/core).  Each core:
  - streams its hidden shard (f32) on the two HWDGE rings (SP + Act)
  - dma_gather's per-sample center rows from a host-precast bf16 table
    (gathers ride 4 SWDGE queues, single_packet descriptor bursts)
  - per 128-sample tile: DVE subtract, ACT square+accumulate -> dist
  - bins (dist, 1) per class with a hi/lo class-id split: two tiny
    one-hots [128,8]/[128,128] and one PE matmul accumulating a
    [16,128] f32 PSUM tile (rows 0:8 = D bins, rows 8:16 = hist bins)
Host combines the 8 per-core [16,128] partials: s = sum D/(hist+1),
loss = 0.5*s/(s+eps).
"""

import numpy as np

B = 65536
D = 512
C = 1000
NCORES = 8
BLOC = B // NCORES          # 8192 samples per core
P = 128                     # partitions
SLOTS = BLOC // P           # 64 sample-slots per partition
PLAN = [(i * 4, 4) for i in range(16)]  # (start_slot, n_slots)
CHUNKS = len(PLAN)
QPC = 4                     # max slots per chunk (tile sizing)
HI = 8                      # class-id high part (c >> 7), 0..7
LO = 128                    # class-id low part (c & 127)
LAMBDA_C = 1.0
EPS = 1e-6

_CACHE = {}


def _build_program():
    import concourse.bacc as bacc
    import concourse.tile as tile
    from concourse import library_config, mybir

    f32 = mybir.dt.float32
    i32 = mybir.dt.int32
    f16 = mybir.dt.bfloat16
    f8 = mybir.dt.float8e4
    i16 = mybir.dt.int16
    Alu = mybir.AluOpType
    Act = mybir.ActivationFunctionType

    nc = bacc.Bacc(
        "TRN2",
        target_bir_lowering=False,
        debug=False,
        enable_asserts=False,
        num_devices=NCORES,
        num_swdge_queues=4,
        dynamic_dma_scratch_size=65536,
    )

    hidden_ap = nc.dram_tensor("hidden", [BLOC, D], f32, kind="ExternalInput").ap()
    ctab_ap = nc.dram_tensor("ctab16", [C, D], f8, kind="ExternalInput").ap()
    # ypg(64) | clslo(128) | clshi(8), one DMA for all the small inputs
    pk_ap = nc.dram_tensor("pk", [P, 200], i32, kind="ExternalInput").ap()
    yidx_ap = nc.dram_tensor("yidx", [P, 8 * SLOTS], i16, kind="ExternalInput").ap()
    out_ap = nc.dram_tensor("out", [2 * HI, LO], f32, kind="ExternalOutput").ap()

    # sample (p, t) of this core's shard is shard row p*SLOTS + t
    hview = hidden_ap.rearrange("(p t) d -> p t d", p=P)

    with tile.TileContext(nc) as tc:
        with (
            tc.tile_pool(name="persist", bufs=1) as persist,
            tc.tile_pool(name="hpool", bufs=5) as hpool,
            tc.tile_pool(name="hodd", bufs=1) as hodd,
            tc.tile_pool(name="cpool", bufs=6) as cpool,
            tc.tile_pool(name="dpool", bufs=2) as dpool,
            tc.tile_pool(name="spool", bufs=4) as spool,
            tc.tile_pool(name="lpool", bufs=4) as lpool,
            tc.tile_pool(name="opool", bufs=4) as opool,
            tc.tile_pool(name="psum", bufs=1, space="PSUM") as psum,
        ):
            nc.gpsimd.load_library(library_config.mlp)
            # small inputs first on the SP ring so they are at the FIFO
            # front, ahead of the bulk hidden stream
            pk = persist.tile([P, 200], i32)
            nc.sync.dma_start(pk[:], pk_ap[:])
            yidx = persist.tile([P, 8 * SLOTS], i16)
            nc.sync.dma_start(yidx[:], yidx_ap[:])

            # y -> (hi, lo) split, as f16 per-partition scalar sources
            ylo_i = persist.tile([P, SLOTS], i32)
            nc.vector.tensor_scalar(ylo_i[:], pk[:, :SLOTS], 127, None, op0=Alu.bitwise_and)
            yhi_i = persist.tile([P, SLOTS], i32)
            nc.vector.tensor_scalar(
                yhi_i[:], pk[:, :SLOTS], 7, None, op0=Alu.logical_shift_right
            )
            ylo = persist.tile([P, SLOTS], f16)
            nc.vector.tensor_copy(ylo[:], ylo_i[:])
            yhi = persist.tile([P, SLOTS], f16)
            nc.vector.tensor_copy(yhi[:], yhi_i[:])

            # class-id iotas from the packed host input (exact in bf16)
            cls_lo = persist.tile([P, LO], f16)
            nc.vector.tensor_copy(cls_lo[:], pk[:, SLOTS : SLOTS + LO])
            cls_hi = persist.tile([P, HI], f16)
            nc.vector.tensor_copy(cls_hi[:], pk[:, SLOTS + LO :])

            dist = persist.tile([P, SLOTS], f32)
            acc = psum.tile([2 * HI, LO], f32)

            # odd-chunk hidden loads ride the Activation ring; issue them all
            # up front so they sit ahead of the SQUARE stream in that
            # sequencer (in-order sequencers!)
            hodd_tiles = {}
            for g, (s0, ns) in enumerate(PLAN):
                if g % 2 == 1:
                    t = hodd.tile([P, QPC, D], f32, name=f"hodd_{g}")
                    nc.scalar.dma_start(t[:, :ns, :], hview[:, s0 : s0 + ns, :])
                    hodd_tiles[g] = t

            for g, (s0, ns) in enumerate(PLAN):
                sl = slice(s0, s0 + ns)
                if g % 2 == 0:
                    h32 = hpool.tile([P, QPC, D], f32)
                    nc.sync.dma_start(h32[:, :ns, :], hview[:, sl, :])
                else:
                    h32 = hodd_tiles[g]
                c16 = cpool.tile([P, QPC, D], f8)
                nc.gpsimd.dma_gather(
                    c16[:, :ns, :],
                    ctab_ap[:],
                    yidx[:, 8 * s0 : 8 * (s0 + ns)],
                    num_idxs=P * ns,
                    num_idxs_reg=P * ns,
                    elem_size=D,
                    single_packet=True,
                    queue_num=g % 4,
                )

                # whole-chunk subtract (one DVE op); h f32, c bf16, out bf16
                diff = dpool.tile([P, QPC, D], f16)
                nc.vector.tensor_sub(
                    diff[:, :ns, :].rearrange("p q d -> p (q d)"),
                    h32[:, :ns, :].rearrange("p q d -> p (q d)"),
                    c16[:, :ns, :].rearrange("p q d -> p (q d)"),
                )

                # per-tile square + accumulate -> dist column (ACT)
                for q in range(ns):
                    t = s0 + q
                    sq = spool.tile([P, D], f16)
                    nc.scalar.activation(
                        sq[:], diff[:, q, :], Act.Square,
                        accum_out=dist[:, t : t + 1],
                    )

                # batched one-hots for the whole chunk
                lhsT = lpool.tile([P, QPC, 2 * HI], f16)
                nc.vector.tensor_tensor(
                    lhsT[:, :ns, HI:],
                    cls_hi[:].unsqueeze(1).to_broadcast([P, ns, HI]),
                    yhi[:, sl].unsqueeze(2).to_broadcast([P, ns, HI]),
                    op=Alu.is_equal,
                )
                nc.vector.tensor_tensor(
                    lhsT[:, :ns, :HI],
                    lhsT[:, :ns, HI:],
                    dist[:, sl].unsqueeze(2).to_broadcast([P, ns, HI]),
                    op=Alu.mult,
                )
                ohlo = opool.tile([P, QPC, LO], f16)
                nc.vector.tensor_tensor(
                    ohlo[:, :ns, :],
                    cls_lo[:].unsqueeze(1).to_broadcast([P, ns, LO]),
                    ylo[:, sl].unsqueeze(2).to_broadcast([P, ns, LO]),
                    op=Alu.is_equal,
                )

                for q in range(ns):
                    t = s0 + q
                    nc.tensor.matmul(
                        out=acc[:],
                        lhsT=lhsT[:, q, :],
                        rhs=ohlo[:, q, :],
                        start=(t == 0),
                        stop=(t == SLOTS - 1),
                    )

            res = persist.tile([2 * HI, LO], f32)
            nc.vector.tensor_copy(res[:], acc[:])
            nc.sync.dma_start(out_ap[:], res[:])

    nc.compile()
    return nc


def _prep_core_inputs(y_shard, hidden_shard, ctab16):
    """Host-side layout marshaling for one core's shard."""
    ypg = y_shard.astype(np.int32).reshape(P, SLOTS)  # sample (p,t) = row p*SLOTS+t

    # dma_gather idx list for chunk g, position j = q*128 + p -> sample
    # (p, g*QPC+q); wrapped: idx j lives at [j % 16, j // 16], replicated
    # over the 8 groups of 16 partitions.
    cols = []
    for s0, ns in PLAN:
        flat = ypg[:, s0 : s0 + ns].T.reshape(P * ns)
        wrapped = flat.reshape(P * ns // 16, 16).T
        cols.append(np.tile(wrapped, (P // 16, 1)))
    yidx = np.concatenate(cols, axis=1).astype(np.int16)

    pk = np.empty((P, 200), dtype=np.int32)
    pk[:, :SLOTS] = ypg
    pk[:, SLOTS : SLOTS + LO] = np.arange(LO, dtype=np.int32)
    pk[:, SLOTS + LO :] = np.arange(HI, dtype=np.int32)

    return {
        "hidden": np.ascontiguousarray(hidden_shard, dtype=np.float32),
        "ctab16": ctab16,
        "pk": pk,
        "yidx": np.ascontiguousarray(yidx),
    }


def combine_partials(outs):
    """outs: list of [16, 128] f32 per core -> scalar loss (f32)."""
    total = np.zeros((2 * HI, LO), dtype=np.float64)
    for o in outs:
        total += o.astype(np.float64)
    Dbins = total[:HI].reshape(HI * LO)[:C]
    hist = total[HI:].reshape(HI * LO)[:C]
    s = float(np.sum(Dbins / (hist + 1.0)))
    try:
        # match the reference's XLA f32 division rounding exactly
        import jax.numpy as jnp

        s32 = jnp.float32(s)
        loss = jnp.float32(LAMBDA_C / 2.0) * s32 / (s32 + jnp.float32(EPS))
        return np.asarray(loss, dtype=np.float32)
    except Exception:
        return np.float32((LAMBDA_C / 2.0) * s / (s + EPS))


def kernel(y, hidden, centers):
    import ml_dtypes
    from concourse.bass_utils import run_bass_kernel_spmd

    y = np.asarray(y).astype(np.int32)
    hidden = np.asarray(hidden, dtype=np.float32)
    centers = np.asarray(centers, dtype=np.float32)
    ctab16 = np.ascontiguousarray(centers.astype(ml_dtypes.float8_e4m3))

    if "nc" not in _CACHE:
        _CACHE["nc"] = _build_program()
    nc = _CACHE["nc"]

    in_maps = [
        _prep_core_inputs(
            y[k * BLOC : (k + 1) * BLOC],
            hidden[k * BLOC : (k + 1) * BLOC],
            ctab16,
        )
        for k in range(NCORES)
    ]

    res = run_bass_kernel_spmd(nc, in_maps, core_ids=list(range(NCORES)))
    outs = [r["out"] for r in res.results]
    return combine_partials(outs)
